# revision 46
# baseline (speedup 1.0000x reference)
"""MQA attention kernel for Trainium2, 8 NeuronCores.

Problem: q,kv [2,2048,1024]; w_q [1024,1024]; w_kv [1024,128]; w_concat
[1024,1024]; 16 heads, d_head 64, shared single K/V head (MQA).

Sharding: queries over L. Flatten (B,L) -> 4096 query rows; core c handles
batch b=c//4, rows (c%4)*512..+512. Each core computes the full 16-head
attention for its 512 query rows against the full 2048 keys of its batch,
then its rows of out @ w_concat. Output rows are disjoint -> no collective.

Per-core layouts (host pre-transposes, zero device cost):
  qT   [1024, 512]  = q_rows.T          (d_model on partitions)
  kvT  [1024, 2048] = kv[b].T
  w_q scaled by 1/8 on host (power of two, exact) -> scores pre-scaled.

Device pipeline (matmuls in fp32r = full PE rate at N>=256; softmax exp on
ACT is the bottleneck engine, everything else is arranged to hide under it):
  - input DMAs emitted in priority order (kv block 0, q, first w_q columns
    first) so the first attention block starts ~15us in; kvT streams
    through a small recycling pool instead of full SBUF residency.
  - kvpT [128,2048] = w_kv.T @ kvT per 512-col block; kkT holds k.T
    duplicated on both partition halves so paired heads' score matmuls use
    complementary PE row groups (concurrent on HW). v_aug[128,16,65] =
    PE-transposed v chunks + ones column (accumulates softmax sums).
  - qpT groups (w_q.T @ qT) are spread one matmul per attention block into
    the PE slack under ACT.
  - per head pair, per key-chunk pair: scores.T [128,1024] PSUM -> ACT exp
    -> SBUF -> out_augT [65,512] += v_aug.T @ exp (row 64 = sums).
  - normalize: accumulator copied to SBUF early (frees PSUM bank), DVE
    reciprocal, GPSIMD partition-broadcast, DVE multiply.
  - final [512,1024] = outT.T @ w_concat -> SBUF -> DRAM.
Cross-partition moves (k.T duplication, odd-head outputs) go via GPSIMD
SWDGE DMA so they never queue behind the input stream on HWDGE.
"""

import numpy as np

B, L, DM = 2, 2048, 1024
H, DH = 16, 64
NCORES = 8
QR = 512          # query rows per core
KC = 16           # key chunks of 128
P = 128

_CACHE = {}


def _build_bass():
    import concourse.mybir as mybir
    import concourse.tile as tile
    from concourse import bacc
    from concourse.masks import make_identity

    f32 = mybir.dt.float32
    f32r = mybir.dt.float32r
    Exp = mybir.ActivationFunctionType.Exp

    nc = bacc.Bacc(
        "TRN2", target_bir_lowering=False, debug=False, enable_asserts=True
    )

    qT = nc.dram_tensor("qT", [DM, QR], f32r, kind="ExternalInput").ap()
    kvT = nc.dram_tensor("kvT", [DM, L], f32r, kind="ExternalInput").ap()
    # wq / wkv arrive host-pre-tiled so every DMA is a contiguous linear
    # copy (>=2KB per partition line) instead of 512B strided reads
    wq = nc.dram_tensor("wq", [8, P, 8, P], f32r, kind="ExternalInput").ap()
    wkv = nc.dram_tensor(
        "wkv", [P, 8, 2 * DH], f32r, kind="ExternalInput"
    ).ap()
    wc = nc.dram_tensor("wc", [DM, DM], f32r, kind="ExternalInput").ap()
    vones = nc.dram_tensor("vones", [P, KC], f32r, kind="ExternalInput").ap()
    out = nc.dram_tensor("out", [QR, DM], f32, kind="ExternalOutput").ap()

    with tile.TileContext(nc) as tc:
        with (
            tc.tile_pool(name="persist", bufs=1) as persist,
            tc.tile_pool(name="kvs", bufs=9) as kvs,
            tc.tile_pool(name="work", bufs=3) as work,
            tc.tile_pool(name="sc_ps", bufs=3, space="PSUM") as sc_ps,
            tc.tile_pool(name="av_ps", bufs=2, space="PSUM") as av_ps,
        ):
            qpT = persist.tile([P, 8, QR], f32r, name="qpT")
            kvpT = persist.tile([P, L], f32, name="kvpT")
            kkT = persist.tile([P, L], f32r, name="kkT")
            v_aug = persist.tile([P, KC, DH + 1], f32r, name="v_aug")
            outT = [
                persist.tile([P, QR], f32r, name=f"outT{pp}")
                for pp in range(8)
            ]
            identf = persist.tile([P, DH], f32, name="identf")
            wkv_sb = persist.tile([P, 8, 2 * DH], f32r, name="wkv_sb")
            wq_sb = [
                persist.tile([P, 8, P], f32r, name=f"wq_sb{mt}")
                for mt in range(8)
            ]
            qT_sb = persist.tile([P, 8, QR], f32r, name="qT_sb")
            wc_sb = persist.tile([P, 8, DM], f32r, name="wc_sb")

            ident = identf[DH : 2 * DH, :]  # base partition 64, matches v rows
            make_identity(nc, ident)

            # ---- input DMAs, in priority order ----
            nc.sync.dma_start(wkv_sb, wkv)
            nc.sync.dma_start(v_aug[:, :, DH], vones)
            kv_chunks = {}

            def dma_kv_block(nt):
                sl = slice(nt * QR, (nt + 1) * QR)
                for kt in range(8):
                    ch = kvs.tile([P, QR], f32r, tag="kv", name="kv_ch")
                    nc.sync.dma_start(ch, kvT[kt * P : (kt + 1) * P, sl])
                    kv_chunks[(nt, kt)] = ch

            def dma_wq_cols(mt):
                nc.sync.dma_start(wq_sb[mt], wq[mt])

            dma_kv_block(0)
            nc.sync.dma_start(
                qT_sb, qT.rearrange("(k p) m -> p k m", p=P)
            )
            dma_wq_cols(0)
            dma_wq_cols(1)
            dma_kv_block(1)
            dma_wq_cols(2)
            dma_wq_cols(3)
            dma_kv_block(2)
            dma_wq_cols(4)
            dma_wq_cols(5)
            dma_kv_block(3)
            dma_wq_cols(6)
            dma_wq_cols(7)
            for kt in range(8):
                nc.sync.dma_start(
                    wc_sb[:, kt, :], wc[kt * P : (kt + 1) * P, :]
                )

            # ---- kv projection per 512-col block (emitted pipelined:
            # nt=0 upfront, nt=1..3 interleaved into pair 0 below so the
            # in-order PE never waits on a far-future kv DMA block) ----
            def kv_block(nt):
                sl = slice(nt * QR, (nt + 1) * QR)
                ps = sc_ps.tile([P, 1024], f32, tag="sc", name="ps_kv")[
                    :, 0:QR
                ]
                for kt in range(8):
                    nc.tensor.matmul(
                        ps,
                        wkv_sb[:, kt, :],
                        kv_chunks.pop((nt, kt)),
                        start=(kt == 0),
                        stop=(kt == 7),
                    )
                nc.vector.tensor_copy(kvpT[:, sl], ps)
                nc.vector.tensor_copy(kkT[0:DH, sl], ps[0:DH, :])
                # duplicate k.T into rows 64:128 (partition shift; SWDGE so
                # it doesn't queue behind the input stream)
                nc.gpsimd.dma_start(kkT[DH : 2 * DH, sl], kkT[0:DH, sl])
                for kc in range(nt * 4, nt * 4 + 4):
                    tp = sc_ps.tile([P, 1024], f32, tag="sc", name="tp")[
                        :, 0:DH
                    ]
                    nc.tensor.transpose(
                        tp, kvpT[DH : 2 * DH, kc * P : (kc + 1) * P], ident
                    )
                    nc.vector.tensor_copy(v_aug[:, kc, 0:DH], tp)

            kv_block(0)

            # ---- q projection: groups 0,1 upfront; 2..7 spread below ----
            qp_ps = {}

            def qp_mm(mt, kt):
                if kt == 0:
                    qp_ps[mt] = sc_ps.tile(
                        [P, 1024], f32, tag="sc", name="ps_q"
                    )[:, 0:QR]
                nc.tensor.matmul(
                    qp_ps[mt],
                    wq_sb[mt][:, kt, :],
                    qT_sb[:, kt, :],
                    start=(kt == 0),
                    stop=(kt == 7),
                )
                if kt == 7:
                    nc.vector.tensor_copy(qpT[:, mt, :], qp_ps.pop(mt))

            for mt in range(2):
                for kt in range(8):
                    qp_mm(mt, kt)

            # ---- attention: head pairs on complementary PE row groups ----
            for p in range(8):
                avps = [
                    av_ps.tile([DH + 1, QR], f32, tag="av", name="avp")
                    for _ in range(2)
                ]
                for kb in range(8):
                    if p == 0 and kb in (2, 4, 6):
                        kv_block(kb // 2)  # pipeline kv projection blocks
                    if p + 2 < 8:
                        qp_mm(p + 2, kb)  # hide q projection in ACT slack
                    scps = [
                        sc_ps.tile([P, 1024], f32, tag="sc", name="scp")
                        for _ in range(2)
                    ]
                    for j in range(2):
                        kc = kb * 2 + j
                        for h2 in range(2):
                            lo = h2 * DH
                            nc.tensor.matmul(
                                scps[h2][:, j * QR : (j + 1) * QR],
                                kkT[lo : lo + DH, kc * P : (kc + 1) * P],
                                qpT[lo : lo + DH, p, :],
                                start=True,
                                stop=True,
                            )
                    es = []
                    for h2 in range(2):
                        e = work.tile([P, 1024], f32r, tag="e", name="e")
                        nc.scalar.activation(e, scps[h2], Exp)
                        es.append(e)
                    for j in range(2):
                        kc = kb * 2 + j
                        for h2 in range(2):
                            nc.tensor.matmul(
                                avps[h2],
                                v_aug[:, kc, :],
                                es[h2][:, j * QR : (j + 1) * QR],
                                start=(kc == 0),
                                stop=(kc == KC - 1),
                            )
                for h2 in range(2):
                    if p < 7:
                        # copy accumulator out of PSUM (frees the bank for
                        # the next pair), normalize from SBUF
                        src = work.tile(
                            [DH + 1, QR], f32, tag="avsb", name="avsb",
                            bufs=2,
                        )
                        nc.vector.tensor_copy(src, avps[h2])
                    else:
                        src = avps[h2]  # last pair: shortest chain wins
                    rcp64 = work.tile(
                        [P, QR], f32, tag="rcp64", name="rcp64", bufs=2
                    )
                    nc.vector.reciprocal(
                        rcp64[DH : DH + 1, :], src[DH : DH + 1, :]
                    )
                    # shift the reciprocal row to partition 0: the
                    # partition_broadcast ucode reads partition 0
                    rcp0 = work.tile([1, QR], f32, tag="rcp0", name="rcp0",
                                     bufs=2)
                    nc.gpsimd.dma_start(rcp0, rcp64[DH : DH + 1, :])
                    bcs = work.tile(
                        [DH, QR], f32, tag="bcs", name="bcs", bufs=2
                    )
                    nc.gpsimd.partition_broadcast(bcs, rcp0)
                    if h2 == 0:
                        nc.vector.tensor_mul(
                            out=outT[p][0:DH, :], in0=src[0:DH, :], in1=bcs
                        )
                    else:
                        otmp = work.tile(
                            [DH, QR], f32r, tag="otmp", name="otmp", bufs=2
                        )
                        nc.vector.tensor_mul(
                            out=otmp, in0=src[0:DH, :], in1=bcs
                        )
                        nc.gpsimd.dma_start(outT[p][DH : 2 * DH, :], otmp)

            # ---- final = outT.T @ w_concat -> [512, 1024] ----
            for mq in range(4):
                for n in range(2):
                    fp = sc_ps.tile([P, 1024], f32, tag="sc", name="fp")[
                        :, 0:QR
                    ]
                    for kt in range(8):
                        nc.tensor.matmul(
                            fp,
                            outT[kt][:, mq * P : (mq + 1) * P],
                            wc_sb[:, kt, n * QR : (n + 1) * QR],
                            start=(kt == 0),
                            stop=(kt == 7),
                        )
                    fsb = work.tile(
                        [P, QR], f32, tag="fsb", name="fsb", bufs=2
                    )
                    nc.vector.tensor_copy(fsb, fp)
                    nc.sync.dma_start(
                        out[mq * P : (mq + 1) * P, n * QR : (n + 1) * QR],
                        fsb,
                    )

    nc.compile()
    return nc


def _get_nc():
    if "nc" not in _CACHE:
        _CACHE["nc"] = _build_bass()
    return _CACHE["nc"]


def make_in_maps(q, kv, w_q, w_kv, w_concat):
    q = np.asarray(q, np.float32)
    kv = np.asarray(kv, np.float32)
    w_qs = (np.asarray(w_q, np.float32) * 0.125).astype(np.float32)
    w_kv = np.asarray(w_kv, np.float32)
    w_concat = np.asarray(w_concat, np.float32)

    kvT = [np.ascontiguousarray(kv[b].T) for b in range(B)]
    # pre-tile weights to the exact SBUF layouts (pure linear DMAs):
    # wq_t[mt, p, kt, m] = w_qs[kt*128+p, mt*128+m]
    wq_t = np.ascontiguousarray(
        w_qs.reshape(8, P, 8, P).transpose(2, 1, 0, 3)
    )
    # wkv_t[p, kt, m] = w_kv[kt*128+p, m]
    wkv_t = np.ascontiguousarray(
        w_kv.reshape(8, P, 2 * DH).transpose(1, 0, 2)
    )
    in_maps = []
    for c in range(NCORES):
        b, s = c // 4, (c % 4) * QR
        in_maps.append(
            {
                "qT": np.ascontiguousarray(q[b, s : s + QR, :].T),
                "kvT": kvT[b],
                "wq": wq_t,
                "wkv": wkv_t,
                "wc": w_concat,
                "vones": np.ones((P, KC), np.float32),
            }
        )
    return in_maps


def assemble(results):
    full = np.empty((B, L, DM), np.float32)
    for c in range(NCORES):
        b, s = c // 4, (c % 4) * QR
        full[b, s : s + QR, :] = results[c]["out"]
    return full


def kernel(q, kv, w_q, w_kv, w_concat):
    from concourse.bass_utils import run_bass_kernel_spmd

    nc = _get_nc()
    in_maps = make_in_maps(q, kv, w_q, w_kv, w_concat)
    res = run_bass_kernel_spmd(nc, in_maps, core_ids=list(range(NCORES)))
    return assemble(res.results)


# revision 52
# speedup vs baseline: 1.0247x; 1.0247x over previous
"""MQA attention kernel for Trainium2, 8 NeuronCores.

Problem: q,kv [2,2048,1024]; w_q [1024,1024]; w_kv [1024,128]; w_concat
[1024,1024]; 16 heads, d_head 64, shared single K/V head (MQA).

Sharding: queries over L. Flatten (B,L) -> 4096 query rows; core c handles
batch b=c//4, rows (c%4)*512..+512. Each core computes the full 16-head
attention for its 512 query rows against the full 2048 keys of its batch,
then its rows of out @ w_concat. Output rows are disjoint -> no collective.

Per-core layouts (host pre-transposes, zero device cost):
  qT   [1024, 512]  = q_rows.T          (d_model on partitions)
  kvT  [1024, 2048] = kv[b].T
  w_q scaled by 1/8 on host (power of two, exact) -> scores pre-scaled.

Device pipeline (matmuls in fp32r = full PE rate at N>=256; softmax exp on
ACT is the bottleneck engine, everything else is arranged to hide under it):
  - input DMAs emitted in priority order (kv block 0, q, first w_q columns
    first) so the first attention block starts ~15us in; kvT streams
    through a small recycling pool instead of full SBUF residency.
  - kvpT [128,2048] = w_kv.T @ kvT per 512-col block; kkT holds k.T
    duplicated on both partition halves so paired heads' score matmuls use
    complementary PE row groups (concurrent on HW). v_aug[128,16,65] =
    PE-transposed v chunks + ones column (accumulates softmax sums).
  - qpT groups (w_q.T @ qT) are spread one matmul per attention block into
    the PE slack under ACT.
  - per head pair, per key-chunk pair: scores.T [128,1024] PSUM -> ACT exp
    -> SBUF -> out_augT [65,512] += v_aug.T @ exp (row 64 = sums).
  - normalize: accumulator copied to SBUF early (frees PSUM bank), DVE
    reciprocal, GPSIMD partition-broadcast, DVE multiply.
  - final [512,1024] = outT.T @ w_concat -> SBUF -> DRAM.
Cross-partition moves (k.T duplication, odd-head outputs) go via GPSIMD
SWDGE DMA so they never queue behind the input stream on HWDGE.
"""

import numpy as np

B, L, DM = 2, 2048, 1024
H, DH = 16, 64
NCORES = 8
QR = 512          # query rows per core
KC = 16           # key chunks of 128
P = 128

_CACHE = {}


def _build_bass():
    import concourse.mybir as mybir
    import concourse.tile as tile
    from concourse import bacc
    from concourse.masks import make_identity

    f32 = mybir.dt.float32
    f32r = mybir.dt.float32r
    Exp = mybir.ActivationFunctionType.Exp

    nc = bacc.Bacc(
        "TRN2", target_bir_lowering=False, debug=False, enable_asserts=True
    )

    qT = nc.dram_tensor("qT", [DM, QR], f32r, kind="ExternalInput").ap()
    kvT = nc.dram_tensor("kvT", [DM, L], f32r, kind="ExternalInput").ap()
    # wq / wkv arrive host-pre-tiled so every DMA is a contiguous linear
    # copy (>=2KB per partition line) instead of 512B strided reads
    wq = nc.dram_tensor("wq", [8, P, 8, P], f32r, kind="ExternalInput").ap()
    wkv = nc.dram_tensor(
        "wkv", [P, 8, 2 * DH], f32r, kind="ExternalInput"
    ).ap()
    wc = nc.dram_tensor("wc", [DM, DM], f32r, kind="ExternalInput").ap()
    vones = nc.dram_tensor("vones", [P, KC], f32r, kind="ExternalInput").ap()
    out = nc.dram_tensor("out", [QR, DM], f32, kind="ExternalOutput").ap()

    with tile.TileContext(nc) as tc:
        with (
            tc.tile_pool(name="persist", bufs=1) as persist,
            tc.tile_pool(name="kvs", bufs=9) as kvs,
            tc.tile_pool(name="work", bufs=3) as work,
            tc.tile_pool(name="sc_ps", bufs=3, space="PSUM") as sc_ps,
            tc.tile_pool(name="av_ps", bufs=2, space="PSUM") as av_ps,
        ):
            qpT = persist.tile([P, 8, QR], f32r, name="qpT")
            kvpT = persist.tile([P, L], f32, name="kvpT")
            kkT = persist.tile([P, L], f32r, name="kkT")
            v_aug = persist.tile([P, KC, DH + 1], f32r, name="v_aug")
            outT = [
                persist.tile([P, QR], f32r, name=f"outT{pp}")
                for pp in range(8)
            ]
            identf = persist.tile([P, DH], f32, name="identf")
            wkv_sb = persist.tile([P, 8, 2 * DH], f32r, name="wkv_sb")
            wq_sb = [
                persist.tile([P, 8, P], f32r, name=f"wq_sb{mt}")
                for mt in range(8)
            ]
            qT_sb = persist.tile([P, 8, QR], f32r, name="qT_sb")
            wc_sb = persist.tile([P, 8, DM], f32r, name="wc_sb")

            ident = identf[DH : 2 * DH, :]  # base partition 64, matches v rows
            make_identity(nc, ident)

            # ---- input DMAs, in priority order ----
            nc.sync.dma_start(wkv_sb, wkv)
            nc.sync.dma_start(v_aug[:, :, DH], vones)
            kv_chunks = {}

            def dma_kv_block(nt):
                sl = slice(nt * QR, (nt + 1) * QR)
                for kt in range(8):
                    ch = kvs.tile([P, QR], f32r, tag="kv", name="kv_ch")
                    nc.sync.dma_start(ch, kvT[kt * P : (kt + 1) * P, sl])
                    kv_chunks[(nt, kt)] = ch

            def dma_wq_cols(mt):
                nc.sync.dma_start(wq_sb[mt], wq[mt])

            dma_kv_block(0)
            nc.sync.dma_start(
                qT_sb, qT.rearrange("(k p) m -> p k m", p=P)
            )
            dma_wq_cols(0)
            dma_wq_cols(1)
            dma_kv_block(1)
            dma_wq_cols(2)
            dma_wq_cols(3)
            dma_kv_block(2)
            dma_wq_cols(4)
            dma_wq_cols(5)
            dma_kv_block(3)
            dma_wq_cols(6)
            dma_wq_cols(7)
            for kt in range(8):
                nc.sync.dma_start(
                    wc_sb[:, kt, :], wc[kt * P : (kt + 1) * P, :]
                )

            # ---- kv projection per 512-col block (emitted pipelined:
            # nt=0 upfront, nt=1..3 interleaved into pair 0 below so the
            # in-order PE never waits on a far-future kv DMA block) ----
            def kv_block(nt):
                sl = slice(nt * QR, (nt + 1) * QR)
                ps = sc_ps.tile([P, 1024], f32, tag="sc", name="ps_kv")[
                    :, 0:QR
                ]
                for kt in range(8):
                    nc.tensor.matmul(
                        ps,
                        wkv_sb[:, kt, :],
                        kv_chunks.pop((nt, kt)),
                        start=(kt == 0),
                        stop=(kt == 7),
                    )
                nc.vector.tensor_copy(kvpT[:, sl], ps)
                nc.vector.tensor_copy(kkT[0:DH, sl], ps[0:DH, :])
                # duplicate k.T into rows 64:128 (partition shift; SWDGE so
                # it doesn't queue behind the input stream)
                nc.gpsimd.dma_start(kkT[DH : 2 * DH, sl], kkT[0:DH, sl])
                for kc in range(nt * 4, nt * 4 + 4):
                    tp = sc_ps.tile([P, 1024], f32, tag="sc", name="tp")[
                        :, 0:DH
                    ]
                    nc.tensor.transpose(
                        tp, kvpT[DH : 2 * DH, kc * P : (kc + 1) * P], ident
                    )
                    nc.vector.tensor_copy(v_aug[:, kc, 0:DH], tp)

            kv_block(0)

            # ---- q projection: groups 0,1 upfront; 2..7 spread below ----
            qp_ps = {}

            def qp_mm(mt, kt):
                if kt == 0:
                    qp_ps[mt] = sc_ps.tile(
                        [P, 1024], f32, tag="sc", name="ps_q"
                    )[:, 0:QR]
                nc.tensor.matmul(
                    qp_ps[mt],
                    wq_sb[mt][:, kt, :],
                    qT_sb[:, kt, :],
                    start=(kt == 0),
                    stop=(kt == 7),
                )
                if kt == 7:
                    nc.vector.tensor_copy(qpT[:, mt, :], qp_ps.pop(mt))

            for mt in range(2):
                for kt in range(8):
                    qp_mm(mt, kt)

            # ---- attention: head pairs on complementary PE row groups ----
            for p in range(8):
                avps = [
                    av_ps.tile([DH + 1, QR], f32, tag="av", name="avp")
                    for _ in range(2)
                ]
                for kb in range(8):
                    if p == 0 and kb in (2, 4, 6):
                        kv_block(kb // 2)  # pipeline kv projection blocks
                    if p + 2 < 8:
                        qp_mm(p + 2, kb)  # hide q projection in ACT slack
                    scps = [
                        sc_ps.tile([P, 1024], f32, tag="sc", name="scp")
                        for _ in range(2)
                    ]
                    for j in range(2):
                        kc = kb * 2 + j
                        for h2 in range(2):
                            lo = h2 * DH
                            nc.tensor.matmul(
                                scps[h2][:, j * QR : (j + 1) * QR],
                                kkT[lo : lo + DH, kc * P : (kc + 1) * P],
                                qpT[lo : lo + DH, p, :],
                                start=True,
                                stop=True,
                            )
                    es = []
                    for h2 in range(2):
                        e = work.tile([P, 1024], f32r, tag="e", name="e")
                        nc.scalar.activation(e, scps[h2], Exp)
                        es.append(e)
                    for j in range(2):
                        kc = kb * 2 + j
                        for h2 in range(2):
                            nc.tensor.matmul(
                                avps[h2],
                                v_aug[:, kc, :],
                                es[h2][:, j * QR : (j + 1) * QR],
                                start=(kc == 0),
                                stop=(kc == KC - 1),
                            )
                for h2 in range(2):
                    if p < 7:
                        # copy accumulator out of PSUM (frees the bank for
                        # the next pair), normalize from SBUF
                        src = work.tile(
                            [DH + 1, QR], f32, tag="avsb", name="avsb",
                            bufs=2,
                        )
                        nc.vector.tensor_copy(src, avps[h2])
                    else:
                        src = avps[h2]  # last pair: shortest chain wins
                    rcp64 = work.tile(
                        [P, QR], f32, tag="rcp64", name="rcp64", bufs=2
                    )
                    nc.vector.reciprocal(
                        rcp64[DH : DH + 1, :], src[DH : DH + 1, :]
                    )
                    # shift the reciprocal row to partition 0: the
                    # partition_broadcast ucode reads partition 0
                    rcp0 = work.tile([1, QR], f32, tag="rcp0", name="rcp0",
                                     bufs=2)
                    nc.gpsimd.dma_start(rcp0, rcp64[DH : DH + 1, :])
                    bcs = work.tile(
                        [DH, QR], f32, tag="bcs", name="bcs", bufs=2
                    )
                    nc.gpsimd.partition_broadcast(bcs, rcp0)
                    if h2 == 0:
                        nc.vector.tensor_mul(
                            out=outT[p][0:DH, :], in0=src[0:DH, :], in1=bcs
                        )
                    else:
                        otmp = work.tile(
                            [DH, QR], f32r, tag="otmp", name="otmp", bufs=2
                        )
                        nc.vector.tensor_mul(
                            out=otmp, in0=src[0:DH, :], in1=bcs
                        )
                        nc.gpsimd.dma_start(outT[p][DH : 2 * DH, :], otmp)

            # ---- final = outT.T @ w_concat -> [512, 1024] ----
            # groups 3 and 4 borrow the (just-freed) attention-accumulator
            # banks so five groups can pre-compute their first seven
            # contraction steps while the last pair finishes normalizing,
            # keeping the PE busy (and warm) until outT[7] lands.
            for g in range(8):
                    mq, n = g // 2, g % 2
                    if g in (3, 4):
                        fp = av_ps.tile([P, QR], f32, tag="av", name="fpav")
                    else:
                        fp = sc_ps.tile(
                            [P, 1024], f32, tag="sc", name="fp"
                        )[:, 0:QR]
                    for kt in range(8):
                        nc.tensor.matmul(
                            fp,
                            outT[kt][:, mq * P : (mq + 1) * P],
                            wc_sb[:, kt, n * QR : (n + 1) * QR],
                            start=(kt == 0),
                            stop=(kt == 7),
                        )
                    fsb = work.tile(
                        [P, QR], f32, tag="fsb", name="fsb", bufs=2
                    )
                    nc.vector.tensor_copy(fsb, fp)
                    nc.sync.dma_start(
                        out[mq * P : (mq + 1) * P, n * QR : (n + 1) * QR],
                        fsb,
                    )

    nc.compile()
    return nc


def _get_nc():
    if "nc" not in _CACHE:
        _CACHE["nc"] = _build_bass()
    return _CACHE["nc"]


def make_in_maps(q, kv, w_q, w_kv, w_concat):
    q = np.asarray(q, np.float32)
    kv = np.asarray(kv, np.float32)
    w_qs = (np.asarray(w_q, np.float32) * 0.125).astype(np.float32)
    w_kv = np.asarray(w_kv, np.float32)
    w_concat = np.asarray(w_concat, np.float32)

    kvT = [np.ascontiguousarray(kv[b].T) for b in range(B)]
    # pre-tile weights to the exact SBUF layouts (pure linear DMAs):
    # wq_t[mt, p, kt, m] = w_qs[kt*128+p, mt*128+m]
    wq_t = np.ascontiguousarray(
        w_qs.reshape(8, P, 8, P).transpose(2, 1, 0, 3)
    )
    # wkv_t[p, kt, m] = w_kv[kt*128+p, m]
    wkv_t = np.ascontiguousarray(
        w_kv.reshape(8, P, 2 * DH).transpose(1, 0, 2)
    )
    in_maps = []
    for c in range(NCORES):
        b, s = c // 4, (c % 4) * QR
        in_maps.append(
            {
                "qT": np.ascontiguousarray(q[b, s : s + QR, :].T),
                "kvT": kvT[b],
                "wq": wq_t,
                "wkv": wkv_t,
                "wc": w_concat,
                "vones": np.ones((P, KC), np.float32),
            }
        )
    return in_maps


def assemble(results):
    full = np.empty((B, L, DM), np.float32)
    for c in range(NCORES):
        b, s = c // 4, (c % 4) * QR
        full[b, s : s + QR, :] = results[c]["out"]
    return full


def kernel(q, kv, w_q, w_kv, w_concat):
    from concourse.bass_utils import run_bass_kernel_spmd

    nc = _get_nc()
    in_maps = make_in_maps(q, kv, w_q, w_kv, w_concat)
    res = run_bass_kernel_spmd(nc, in_maps, core_ids=list(range(NCORES)))
    return assemble(res.results)


# revision 56
# speedup vs baseline: 1.0436x; 1.0184x over previous
"""MQA attention kernel for Trainium2, 8 NeuronCores.

Problem: q,kv [2,2048,1024]; w_q [1024,1024]; w_kv [1024,128]; w_concat
[1024,1024]; 16 heads, d_head 64, shared single K/V head (MQA).

Sharding: queries over L. Flatten (B,L) -> 4096 query rows; core c handles
batch b=c//4, rows (c%4)*512..+512. Each core computes the full 16-head
attention for its 512 query rows against the full 2048 keys of its batch,
then its rows of out @ w_concat. Output rows are disjoint -> no collective.

Per-core layouts (host pre-transposes, zero device cost):
  qT   [1024, 512]  = q_rows.T          (d_model on partitions)
  kvT  [1024, 2048] = kv[b].T
  w_q scaled by 1/8 on host (power of two, exact) -> scores pre-scaled.

Device pipeline (matmuls in fp32r = full PE rate at N>=256; softmax exp on
ACT is the bottleneck engine, everything else is arranged to hide under it):
  - input DMAs emitted in priority order (kv block 0, q, first w_q columns
    first) so the first attention block starts ~15us in; kvT streams
    through a small recycling pool instead of full SBUF residency.
  - kvpT [128,2048] = w_kv.T @ kvT per 512-col block; kkT holds k.T
    duplicated on both partition halves so paired heads' score matmuls use
    complementary PE row groups (concurrent on HW). v_aug[128,16,65] =
    PE-transposed v chunks + ones column (accumulates softmax sums).
  - qpT groups (w_q.T @ qT) are spread one matmul per attention block into
    the PE slack under ACT.
  - per head pair, per key-chunk pair: scores.T [128,1024] PSUM -> ACT exp
    -> SBUF -> out_augT [65,512] += v_aug.T @ exp (row 64 = sums).
  - normalize: accumulator copied to SBUF early (frees PSUM bank), DVE
    reciprocal, GPSIMD partition-broadcast, DVE multiply.
  - final [512,1024] = outT.T @ w_concat -> SBUF -> DRAM.
Cross-partition moves (k.T duplication, odd-head outputs) go via GPSIMD
SWDGE DMA so they never queue behind the input stream on HWDGE.
"""

import numpy as np

B, L, DM = 2, 2048, 1024
H, DH = 16, 64
NCORES = 8
QR = 512          # query rows per core
KC = 16           # key chunks of 128
P = 128

_CACHE = {}


def _build_bass():
    import concourse.mybir as mybir
    import concourse.tile as tile
    from concourse import bacc
    from concourse.masks import make_identity

    f32 = mybir.dt.float32
    f32r = mybir.dt.float32r
    Exp = mybir.ActivationFunctionType.Exp

    nc = bacc.Bacc(
        "TRN2", target_bir_lowering=False, debug=False, enable_asserts=True
    )

    qT = nc.dram_tensor("qT", [DM, QR], f32r, kind="ExternalInput").ap()
    kvT = nc.dram_tensor("kvT", [DM, L], f32r, kind="ExternalInput").ap()
    # wq / wkv arrive host-pre-tiled so every DMA is a contiguous linear
    # copy (>=2KB per partition line) instead of 512B strided reads
    wq = nc.dram_tensor("wq", [8, P, 8, P], f32r, kind="ExternalInput").ap()
    wkv = nc.dram_tensor(
        "wkv", [P, 8, 2 * DH], f32r, kind="ExternalInput"
    ).ap()
    wc = nc.dram_tensor("wc", [DM, DM], f32r, kind="ExternalInput").ap()
    vones = nc.dram_tensor("vones", [P, KC], f32r, kind="ExternalInput").ap()
    out = nc.dram_tensor("out", [QR, DM], f32, kind="ExternalOutput").ap()

    with tile.TileContext(nc) as tc:
        with (
            tc.tile_pool(name="persist", bufs=1) as persist,
            tc.tile_pool(name="kvs", bufs=9) as kvs,
            tc.tile_pool(name="work", bufs=3) as work,
            tc.tile_pool(name="sc_ps", bufs=3, space="PSUM") as sc_ps,
            tc.tile_pool(name="av_ps", bufs=2, space="PSUM") as av_ps,
        ):
            qpT = persist.tile([P, 8, QR], f32r, name="qpT")
            kvpT = persist.tile([P, L], f32, name="kvpT")
            kkT = persist.tile([P, L], f32r, name="kkT")
            v_aug = persist.tile([P, KC, DH + 1], f32r, name="v_aug")
            outT = [
                persist.tile([P, QR], f32r, name=f"outT{pp}")
                for pp in range(8)
            ]
            identf = persist.tile([P, DH], f32, name="identf")
            wkv_sb = persist.tile([P, 8, 2 * DH], f32r, name="wkv_sb")
            wq_sb = [
                persist.tile([P, 8, P], f32r, name=f"wq_sb{mt}")
                for mt in range(8)
            ]
            qT_sb = persist.tile([P, 8, QR], f32r, name="qT_sb")
            wc_sb = persist.tile([P, 8, DM], f32r, name="wc_sb")

            ident = identf[DH : 2 * DH, :]  # base partition 64, matches v rows
            make_identity(nc, ident)

            # ---- input DMAs, in priority order ----
            nc.sync.dma_start(wkv_sb, wkv)
            nc.sync.dma_start(v_aug[:, :, DH], vones)
            kv_chunks = {}

            def dma_kv_block(nt):
                sl = slice(nt * QR, (nt + 1) * QR)
                for kt in range(8):
                    ch = kvs.tile([P, QR], f32r, tag="kv", name="kv_ch")
                    nc.sync.dma_start(ch, kvT[kt * P : (kt + 1) * P, sl])
                    kv_chunks[(nt, kt)] = ch

            def dma_wq_cols(mt):
                nc.sync.dma_start(wq_sb[mt], wq[mt])

            dma_kv_block(0)
            nc.sync.dma_start(
                qT_sb, qT.rearrange("(k p) m -> p k m", p=P)
            )
            dma_wq_cols(0)
            dma_wq_cols(1)
            dma_kv_block(1)
            dma_wq_cols(2)
            dma_wq_cols(3)
            dma_kv_block(2)
            dma_wq_cols(4)
            dma_wq_cols(5)
            dma_kv_block(3)
            dma_wq_cols(6)
            dma_wq_cols(7)
            for kt in range(8):
                nc.sync.dma_start(
                    wc_sb[:, kt, :], wc[kt * P : (kt + 1) * P, :]
                )

            # ---- kv projection per 512-col block (emitted pipelined:
            # nt=0 upfront, nt=1..3 interleaved into pair 0 below so the
            # in-order PE never waits on a far-future kv DMA block) ----
            def kv_block(nt):
                sl = slice(nt * QR, (nt + 1) * QR)
                ps = sc_ps.tile([P, 1024], f32, tag="sc", name="ps_kv")[
                    :, 0:QR
                ]
                for kt in range(8):
                    nc.tensor.matmul(
                        ps,
                        wkv_sb[:, kt, :],
                        kv_chunks.pop((nt, kt)),
                        start=(kt == 0),
                        stop=(kt == 7),
                    )
                nc.vector.tensor_copy(kvpT[:, sl], ps)
                nc.vector.tensor_copy(kkT[0:DH, sl], ps[0:DH, :])
                # duplicate k.T into rows 64:128 (partition shift; SWDGE so
                # it doesn't queue behind the input stream)
                nc.gpsimd.dma_start(kkT[DH : 2 * DH, sl], kkT[0:DH, sl])
                for kc in range(nt * 4, nt * 4 + 4):
                    tp = sc_ps.tile([P, 1024], f32, tag="sc", name="tp")[
                        :, 0:DH
                    ]
                    nc.tensor.transpose(
                        tp, kvpT[DH : 2 * DH, kc * P : (kc + 1) * P], ident
                    )
                    nc.vector.tensor_copy(v_aug[:, kc, 0:DH], tp)

            kv_block(0)

            # ---- q projection: groups 0,1 upfront; 2..7 spread below ----
            qp_ps = {}

            def qp_mm(mt, kt):
                if kt == 0:
                    qp_ps[mt] = sc_ps.tile(
                        [P, 1024], f32, tag="sc", name="ps_q"
                    )[:, 0:QR]
                nc.tensor.matmul(
                    qp_ps[mt],
                    wq_sb[mt][:, kt, :],
                    qT_sb[:, kt, :],
                    start=(kt == 0),
                    stop=(kt == 7),
                )
                if kt == 7:
                    nc.vector.tensor_copy(qpT[:, mt, :], qp_ps.pop(mt))

            for mt in range(2):
                for kt in range(8):
                    qp_mm(mt, kt)

            # ---- attention: head pairs on complementary PE row groups ----
            for p in range(8):
                avps = [
                    av_ps.tile([DH + 1, QR], f32, tag="av", name="avp")
                    for _ in range(2)
                ]
                for kb in range(8):
                    if p == 0 and kb in (2, 4, 6):
                        kv_block(kb // 2)  # pipeline kv projection blocks
                    if p + 2 < 8:
                        qp_mm(p + 2, kb)  # hide q projection in ACT slack
                    scps = [
                        sc_ps.tile([P, 1024], f32, tag="sc", name="scp")
                        for _ in range(2)
                    ]
                    for j in range(2):
                        kc = kb * 2 + j
                        for h2 in range(2):
                            lo = h2 * DH
                            nc.tensor.matmul(
                                scps[h2][:, j * QR : (j + 1) * QR],
                                kkT[lo : lo + DH, kc * P : (kc + 1) * P],
                                qpT[lo : lo + DH, p, :],
                                start=True,
                                stop=True,
                            )
                    es = []
                    for h2 in range(2):
                        e = work.tile([P, 1024], f32r, tag="e", name="e")
                        nc.scalar.activation(e, scps[h2], Exp)
                        es.append(e)
                    for j in range(2):
                        kc = kb * 2 + j
                        for h2 in range(2):
                            nc.tensor.matmul(
                                avps[h2],
                                v_aug[:, kc, :],
                                es[h2][:, j * QR : (j + 1) * QR],
                                start=(kc == 0),
                                stop=(kc == KC - 1),
                            )
                for h2 in range(2):
                    if p < 7:
                        # copy accumulator out of PSUM (frees the bank for
                        # the next pair), normalize from SBUF
                        src = work.tile(
                            [DH + 1, QR], f32, tag="avsb", name="avsb",
                            bufs=2,
                        )
                        nc.vector.tensor_copy(src, avps[h2])
                    else:
                        src = avps[h2]  # last pair: shortest chain wins
                    rcp64 = work.tile(
                        [P, QR], f32, tag="rcp64", name="rcp64", bufs=2
                    )
                    nc.vector.reciprocal(
                        rcp64[DH : DH + 1, :], src[DH : DH + 1, :]
                    )
                    # shift the reciprocal row to partition 0: the
                    # partition_broadcast ucode reads partition 0
                    rcp0 = work.tile([1, QR], f32, tag="rcp0", name="rcp0",
                                     bufs=2)
                    nc.gpsimd.dma_start(rcp0, rcp64[DH : DH + 1, :])
                    bcs = work.tile(
                        [DH, QR], f32, tag="bcs", name="bcs", bufs=2
                    )
                    nc.gpsimd.partition_broadcast(bcs, rcp0)
                    if h2 == 0:
                        nc.vector.tensor_mul(
                            out=outT[p][0:DH, :], in0=src[0:DH, :], in1=bcs
                        )
                    else:
                        otmp = work.tile(
                            [DH, QR], f32r, tag="otmp", name="otmp", bufs=2
                        )
                        nc.vector.tensor_mul(
                            out=otmp, in0=src[0:DH, :], in1=bcs
                        )
                        nc.gpsimd.dma_start(outT[p][DH : 2 * DH, :], otmp)

            # ---- final = outT.T @ w_concat -> [512, 1024] ----
            # groups 3 and 4 borrow the (just-freed) attention-accumulator
            # banks so five groups can pre-compute their first seven
            # contraction steps while the last pair finishes normalizing,
            # keeping the PE busy (and warm) until outT[7] lands.
            for g in range(8):
                    mq, n = g // 2, g % 2
                    if g in (3, 4):
                        fp = av_ps.tile([P, QR], f32, tag="av", name="fpav")
                    else:
                        fp = sc_ps.tile(
                            [P, 1024], f32, tag="sc", name="fp"
                        )[:, 0:QR]
                    for kt in range(8):
                        nc.tensor.matmul(
                            fp,
                            outT[kt][:, mq * P : (mq + 1) * P],
                            wc_sb[:, kt, n * QR : (n + 1) * QR],
                            start=(kt == 0),
                            stop=(kt == 7),
                        )
                    fsb = work.tile(
                        [P, QR], f32, tag="fsb", name="fsb", bufs=3
                    )
                    nc.vector.tensor_copy(fsb, fp)
                    nc.sync.dma_start(
                        out[mq * P : (mq + 1) * P, n * QR : (n + 1) * QR],
                        fsb,
                    )

    nc.compile()
    return nc


def _get_nc():
    if "nc" not in _CACHE:
        _CACHE["nc"] = _build_bass()
    return _CACHE["nc"]


def make_in_maps(q, kv, w_q, w_kv, w_concat):
    q = np.asarray(q, np.float32)
    kv = np.asarray(kv, np.float32)
    w_qs = (np.asarray(w_q, np.float32) * 0.125).astype(np.float32)
    w_kv = np.asarray(w_kv, np.float32)
    w_concat = np.asarray(w_concat, np.float32)

    kvT = [np.ascontiguousarray(kv[b].T) for b in range(B)]
    # pre-tile weights to the exact SBUF layouts (pure linear DMAs):
    # wq_t[mt, p, kt, m] = w_qs[kt*128+p, mt*128+m]
    wq_t = np.ascontiguousarray(
        w_qs.reshape(8, P, 8, P).transpose(2, 1, 0, 3)
    )
    # wkv_t[p, kt, m] = w_kv[kt*128+p, m]
    wkv_t = np.ascontiguousarray(
        w_kv.reshape(8, P, 2 * DH).transpose(1, 0, 2)
    )
    in_maps = []
    for c in range(NCORES):
        b, s = c // 4, (c % 4) * QR
        in_maps.append(
            {
                "qT": np.ascontiguousarray(q[b, s : s + QR, :].T),
                "kvT": kvT[b],
                "wq": wq_t,
                "wkv": wkv_t,
                "wc": w_concat,
                "vones": np.ones((P, KC), np.float32),
            }
        )
    return in_maps


def assemble(results):
    full = np.empty((B, L, DM), np.float32)
    for c in range(NCORES):
        b, s = c // 4, (c % 4) * QR
        full[b, s : s + QR, :] = results[c]["out"]
    return full


def kernel(q, kv, w_q, w_kv, w_concat):
    from concourse.bass_utils import run_bass_kernel_spmd

    nc = _get_nc()
    in_maps = make_in_maps(q, kv, w_q, w_kv, w_concat)
    res = run_bass_kernel_spmd(nc, in_maps, core_ids=list(range(NCORES)))
    return assemble(res.results)
